# revision 1
# baseline (speedup 1.0000x reference)
# Trainium2 Bass kernel: batched second-order LPC synthesis
# (frame unfold -> gain -> 11 cascaded biquads -> hann window -> overlap-add -> norm)
#
# Sharding: pure data parallel over batch. 32 batch rows / 8 cores = 4 rows per
# core; each core handles 4*1024 = 4096 frames laid out as 128 partitions x 32
# frame-blocks.
#
# Device algorithm per core:
#  - the 11-section biquad cascade runs as a wavefront over (section, time):
#    wavefront step g updates section s at local time t = g-s+1 for all frames
#    at once, with 3 elementwise ops. Both feedback multiplies are fused in one
#    paired tensor_tensor over the adjacent (t-2, t-1) columns.
#  - state lives in a SKEWED buffer: cell (s, t) sits at column t - 2s + 24 of
#    its frame-block row. A cell is overwritten by (s+1, t+2) exactly 3
#    wavefront steps after being written, and its last reader runs at +2 steps,
#    so the whole 12-slot cascade needs only 536 columns per block — the full
#    512-sample frame runs in ONE wavefront pass (no time chunking, no carry
#    copies). Section 11's outputs land contiguously at columns [2, 514) and
#    are never destroyed.
#  - frame-blocks are statically split between the vector engine and gpsimd,
#    which run independent wavefronts in parallel (separate state tiles).
#  - input staging is double-buffered: unfold DMA -> staging, then one
#    gain-multiply into the skewed slot-0 columns, prefetched ahead.
#  - epilogue: PE transposes 128x128 (frame x time) blocks straight out of the
#    skewed buffer into PSUM; one scalar_tensor_tensor per block applies the
#    hann window (per-partition scalar) and accumulates the overlap-add into an
#    SBUF accumulator.
#  - output: PE transposes the accumulator back to sample-major, a
#    tensor_tensor fuses the 1/norm multiply with the PSUM read-out, and
#    contiguous-row DMAs write the cropped result.
import numpy as np

HOP, WIN, PAD = 128, 512, 192
B, T, S = 32, 131072, 11
F = T // HOP          # 1024
NCORE = 8
NB = B // NCORE       # 4 batch rows per core
L = 64                # input staging chunk
NCHUNK = WIN // L     # 8
NBLK = (NB * F) // 128  # 32 frame blocks per core
NS = 11
CW = WIN + 2 * NS + 2   # 536 skewed columns per block
LFULL = T + 2 * PAD   # 131456
NCELL = LFULL // HOP  # 1027
ACCW = 1028
DB = 22               # frame blocks on DVE; NBLK-DB go to gpsimd
K0S = [1 + 128 * i for i in range(8)] + [898]  # output transpose col bases

_CACHE = {}


def _hann(n):
    return 0.5 * (1.0 - np.cos(2.0 * np.pi * np.arange(n) / n))


def _build_module():
    import concourse.bass as bass
    import concourse.tile as tile
    from concourse import bacc, mybir
    from concourse.ap import AP

    f32 = mybir.dt.float32
    mult = mybir.AluOpType.mult
    add = mybir.AluOpType.add

    nc = bacc.Bacc("TRN2", target_bir_lowering=False, debug=False)
    ex_in = nc.dram_tensor("ex", [NB, T], f32, kind="ExternalInput").ap()
    # coefficients stored section-DESCENDING so every wavefront AP has
    # positive steps: col ((NS-s)*NBLK + b)*2 + {0: c2, 1: c1}
    c21_in = nc.dram_tensor("c21t", [128, NS * NBLK * 2], f32, kind="ExternalInput").ap()
    gb_in = nc.dram_tensor("gb", [128, NBLK * L], f32, kind="ExternalInput").ap()
    win_in = nc.dram_tensor("win4", [128, 4], f32, kind="ExternalInput").ap()
    rnt_in = nc.dram_tensor("rnt", [128, 9 * 128], f32, kind="ExternalInput").ap()
    id_in = nc.dram_tensor("idn", [128, 128], f32, kind="ExternalInput").ap()
    out = nc.dram_tensor("out", [NB, T], f32, kind="ExternalOutput").ap()
    expd = nc.dram_tensor("expd", [NB, LFULL], f32).ap()

    GBN = NBLK - DB  # gpsimd blocks

    with tile.TileContext(nc) as tc:
        with (
            tc.tile_pool(name="state", bufs=1) as st,
            tc.tile_pool(name="scratch", bufs=2) as sp,
            tc.tile_pool(name="psum", bufs=4, space="PSUM") as pp,
        ):
            engines = []
            for nm, eng, nb_e, b0 in (("d", nc.vector, DB, 0),
                                      ("g", nc.gpsimd, GBN, DB)):
                H = st.tile([128, nb_e * CW], f32, tag=f"H{nm}", name=f"H{nm}")
                C21 = st.tile([128, NS * nb_e * 2], f32, tag=f"C21{nm}",
                              name=f"C21{nm}")
                Xs = [st.tile([128, nb_e * L], f32, tag=f"Xs{nm}{h}",
                              name=f"Xs{nm}{h}") for h in range(2)]
                engines.append(dict(nm=nm, eng=eng, nb=nb_e, b0=b0,
                                    H=H, C21=C21, Xs=Xs))

            ACC = st.tile([128, NB * ACCW], f32)
            GBt = st.tile([128, NBLK * L], f32)
            WIN4 = st.tile([128, 4], f32)
            RNT = st.tile([128, 9 * 128], f32)
            IDN = st.tile([128, 128], f32)
            ZER = st.tile([128, 6], f32)

            # one-time loads + init
            for e in engines:
                nb_e, b0 = e["nb"], e["b0"]
                hc = nb_e * CW
                nc.sync.dma_start(
                    e["C21"][:],
                    AP(c21_in.tensor, b0 * 2,
                       [[NS * NBLK * 2, 128], [NBLK * 2, NS], [1, nb_e * 2]]))
                # zero the initial-state cells: cols [0, 22) of each block row
                e["eng"].memset(
                    AP(e["H"][:].tensor, 0,
                       [[hc, 128], [CW, nb_e], [1, 2 * NS]]), 0.0)
            nc.sync.dma_start(GBt[:], gb_in)
            nc.sync.dma_start(WIN4[:], win_in)
            nc.sync.dma_start(RNT[:], rnt_in)
            nc.sync.dma_start(IDN[:], id_in)
            nc.vector.memset(ACC[:], 0.0)
            nc.vector.memset(ZER[:], 0.0)

            # padded excitation in DRAM: expd[:, PAD:PAD+T] = ex, edges 0
            nc.sync.dma_start(
                AP(expd.tensor, PAD, [[LFULL, NB], [1, T]]),
                AP(ex_in.tensor, 0, [[T, NB], [1, T]]))
            nc.sync.dma_start(
                AP(expd.tensor, 0, [[LFULL, NB], [1, PAD]]),
                AP(ZER[:].tensor, 0, [[6, 128], [1, 6]]))
            nc.sync.dma_start(
                AP(expd.tensor, PAD + T, [[LFULL, NB], [1, PAD]]),
                AP(ZER[:].tensor, 0, [[6, 128], [1, 6]]))

            # input staging: unfold DMA -> Xs, gain-multiply -> skewed slot-0
            # (cols t+24). Fresh cells, so arbitrary prefetch is safe; Xs
            # double-buffering paces the DMAs.
            for e in engines:
                nm, eng, nb_e, b0 = e["nm"], e["eng"], e["nb"], e["b0"]
                ht = e["H"][:].tensor
                hc = nb_e * CW
                for ct in range(NCHUNK):
                    xs = e["Xs"][ct % 2]
                    xst = xs[:].tensor
                    b = b0
                    while b < b0 + nb_e:
                        beta = b // 8
                        bhi = min(b0 + nb_e, (beta + 1) * 8)
                        nbb = bhi - b
                        nc.sync.dma_start(
                            AP(xst, (b - b0) * L, [[nb_e * L, 128], [1, nbb * L]]),
                            AP(expd.tensor,
                               beta * LFULL + (b % 8) * 128 * HOP + ct * L,
                               [[HOP, 128], [128 * HOP, nbb], [1, L]]))
                        b = bhi
                    # gain-multiply on gpsimd for both halves: it has
                    # wavefront slack, freeing the busier vector engine
                    nc.gpsimd.tensor_tensor(
                        AP(ht, 24 + ct * L, [[hc, 128], [CW, nb_e], [1, L]]),
                        AP(xst, 0, [[nb_e * L, 128], [L, nb_e], [1, L]]),
                        AP(GBt[:].tensor, b0 * L, [[NBLK * L, 128], [L, nb_e], [1, L]]),
                        op=mult)

            # the single full-length wavefront + trailing epilogue epochs
            for e in engines:
                nm, eng, nb_e = e["nm"], e["eng"], e["nb"]
                ht = e["H"][:].tensor
                c21t = e["C21"][:].tensor
                hc = nb_e * CW
                for g in range(WIN + NS - 1):
                    s_lo = max(1, g - WIN + 2)
                    s_hi = min(NS, g + 1)
                    ns = s_hi - s_lo + 1
                    w = ns * nb_e
                    # APs enumerate s DESCENDING (s_hi..s_lo): all steps +ve.
                    # write col for section s: g - 3s + 25 (block-local)
                    off_w = g - 3 * s_hi + 25
                    pr = sp.tile([128, NS * nb_e * 2], f32, tag=f"pr{nm}",
                                 name=f"pr{nm}")
                    t2 = sp.tile([128, NS * nb_e], f32, tag=f"t2{nm}",
                                 name=f"t2{nm}")
                    pra = pr[:, :2 * w].rearrange("p (s b two) -> p s b two",
                                                  s=ns, two=2)
                    eng.tensor_tensor(
                        pra,
                        AP(ht, off_w - 2, [[hc, 128], [3, ns], [CW, nb_e], [1, 2]]),
                        AP(c21t, (NS - s_hi) * nb_e * 2,
                           [[NS * nb_e * 2, 128], [nb_e * 2, ns], [1, 2 * nb_e]]),
                        op=mult)
                    prt = pr[:].tensor
                    t2a = t2[:, :w].rearrange("p (s b) -> p s b", s=ns)
                    eng.tensor_tensor(
                        t2a,
                        AP(prt, 0, [[NS * nb_e * 2, 128], [2, w]]),
                        AP(prt, 1, [[NS * nb_e * 2, 128], [2, w]]),
                        op=add)
                    eng.tensor_tensor(
                        AP(ht, off_w, [[hc, 128], [3, ns], [CW, nb_e]]),
                        t2a.copy(),
                        AP(ht, off_w + 2, [[hc, 128], [3, ns], [CW, nb_e]]),
                        op=add)

            # overlap-add epilogue: section 11's outputs sit at skewed cols
            # [2, 514) of each block row; transpose 128x128 (frame x time)
            # blocks to PSUM, window + accumulate into ACC
            for j in range(4):
                for b in range(NBLK):
                    e = engines[0] if b < DB else engines[1]
                    bl = b - e["b0"]
                    beta, bb = divmod(b, 8)
                    ps = pp.tile([128, 128], f32, tag="ps", name="ps")
                    nc.tensor.transpose(
                        ps[:],
                        AP(e["H"][:].tensor, bl * CW + 2 + j * 128,
                           [[e["nb"] * CW, 128], [1, 128]]),
                        IDN[:])
                    k0 = beta * ACCW + bb * 128 + j
                    nc.vector.scalar_tensor_tensor(
                        ACC[:, k0:k0 + 128], ps[:], WIN4[:, j:j + 1],
                        ACC[:, k0:k0 + 128], op0=mult, op1=add)

            # output: transpose ACC back to sample-major, multiply 1/norm, DMA
            at = ACC[:].tensor
            for beta in range(NB):
                for i, k0 in enumerate(K0S):
                    ps = pp.tile([128, 128], f32, tag="pso", name="pso")
                    nc.tensor.transpose(
                        ps[:], ACC[:, beta * ACCW + k0:beta * ACCW + k0 + 128],
                        IDN[:])
                    ot = sp.tile([128, 128], f32, tag="ot", name="ot")
                    nc.vector.tensor_tensor(ot[:], ps[:],
                                            RNT[:, i * 128:(i + 1) * 128], op=mult)
                    o_t = ot[:].tensor
                    if i == 0:
                        nc.sync.dma_start(
                            AP(out.tensor, beta * T, [[1, 1], [1, 64]]),
                            AP(o_t, 64, [[128, 1], [1, 64]]))
                        nc.sync.dma_start(
                            AP(out.tensor, beta * T + 64, [[128, 127], [1, 128]]),
                            AP(o_t, 128, [[128, 127], [1, 128]]))
                    elif i < 8:
                        nc.sync.dma_start(
                            AP(out.tensor, beta * T + k0 * 128 - PAD,
                               [[128, 128], [1, 128]]),
                            AP(o_t, 0, [[128, 128], [1, 128]]))
                    else:
                        nc.sync.dma_start(
                            AP(out.tensor, beta * T + 1025 * 128 - PAD, [[1, 1], [1, 64]]),
                            AP(o_t, 127 * 128, [[128, 1], [1, 64]]))

    nc.compile()
    return nc


def _host_prep(ex, gain, biquads):
    # per-core host tiles; frame n = beta*F + f -> p = n % 128, b = n // 128
    f32 = np.float32
    a0 = biquads[..., 0].astype(f32)
    a1 = biquads[..., 1].astype(f32)
    a2 = biquads[..., 2].astype(f32)
    c1 = (-a1 / a0).astype(f32)          # [NB, F, S]
    c2 = (-a2 / a0).astype(f32)
    gain_eff = (gain.astype(f32) * np.prod((1.0 / a0).astype(f32), axis=-1)).astype(f32)

    c1r = c1.reshape(NB, 8, 128, S).transpose(2, 3, 0, 1).reshape(128, S, NBLK)
    c2r = c2.reshape(NB, 8, 128, S).transpose(2, 3, 0, 1).reshape(128, S, NBLK)
    C21 = np.stack([c2r, c1r], axis=-1)[:, ::-1]  # section-descending
    C21 = np.ascontiguousarray(C21.reshape(128, S * NBLK * 2))
    g = gain_eff.reshape(NB, 8, 128).transpose(2, 0, 1).reshape(128, NBLK)
    GB = np.repeat(g[:, :, None], L, axis=2).reshape(128, NBLK * L).astype(f32)
    return C21, np.ascontiguousarray(GB)


def _host_consts():
    f32 = np.float32
    win = _hann(WIN).astype(f32)
    WIN4 = np.ascontiguousarray(win.reshape(4, 128).T)
    norm = np.zeros(LFULL, f32)
    idx = (np.arange(F)[:, None] * HOP + np.arange(WIN)[None, :]).reshape(-1)
    np.add.at(norm, idx, np.broadcast_to(win, (F, WIN)).reshape(-1))
    rn = np.zeros_like(norm)
    nz = norm != 0
    rn[nz] = (f32(1.0) / norm[nz]).astype(f32)
    rn2 = rn.reshape(NCELL, 128)         # [k, p]
    RNT = np.zeros((128, 9 * 128), f32)  # [k_local, i*128 + p]
    for i, k0 in enumerate(K0S):
        RNT[:, i * 128:(i + 1) * 128] = rn2[k0:k0 + 128, :]
    IDN = np.eye(128, dtype=f32)
    return WIN4, RNT, IDN


def kernel(ex, gain, biquads):
    from concourse.bass_utils import run_bass_kernel_spmd

    ex = np.asarray(ex, np.float32)
    gain = np.asarray(gain, np.float32)
    biquads = np.asarray(biquads, np.float32)

    if "nc" not in _CACHE:
        _CACHE["nc"] = _build_module()
    nc = _CACHE["nc"]

    WIN4, RNT, IDN = _host_consts()
    in_maps = []
    for ci in range(NCORE):
        sl = slice(ci * NB, (ci + 1) * NB)
        C21, GB = _host_prep(ex[sl], gain[sl], biquads[sl])
        in_maps.append({
            "ex": np.ascontiguousarray(ex[sl]),
            "c21t": C21, "gb": GB,
            "win4": WIN4, "rnt": RNT, "idn": IDN,
        })
    res = run_bass_kernel_spmd(nc, in_maps, list(range(NCORE)))
    out = np.concatenate([res.results[ci]["out"] for ci in range(NCORE)], axis=0)
    return out.astype(np.float32)



# revision 4
# speedup vs baseline: 1.0912x; 1.0912x over previous
# Trainium2 Bass kernel: batched second-order LPC synthesis
# (frame unfold -> gain -> 11 cascaded biquads -> hann window -> overlap-add -> norm)
#
# Sharding: pure data parallel over batch. 32 batch rows / 8 cores = 4 rows per
# core = 4096 frames, laid out as 128 partitions x 32 frame-blocks.
#
# Device algorithm (v2, "slab ring"):
#  - the 11-section cascade runs as a wavefront over (section, time), but the
#    state is stored in per-step SLABS indexed by tau = t + s - 1: at step g
#    every active section writes time t = g-s+1, and ALL those cells share
#    tau = g+1. A slab is [slot s=0..11][block b] contiguous, so every
#    wavefront operand is a stride-1 run: slot 0 holds the gained input x[t],
#    slots 1..11 hold section outputs.
#  - per step, three ops per chain: pr = (y2-run|y1-run) * (c2-run|c1-run)
#    (one TT over both slabs g-1,g), t2 = pr_lo + pr_hi, y-run = t2 + u-run
#    (u = previous section's output = same slab shifted one slot down).
#  - slabs live in a 16-deep ring (lifetime of a slab is 2 steps).
#  - the DVE state is fp16 (inputs pre-scaled by 1/64; un-scaled in the output
#    norm table): packed stride-1 APs + 2-byte dtype put DVE TT in 2x mode.
#    Two interleaved DVE chains (15+14 blocks) hide the ~95ns semaphore
#    latency between dependent ops. gpsimd (Pool) runs 3 blocks in f32 as an
#    independent third chain.
#  - the scalar (ACT) engine exports section-11 outputs from the ring to a
#    [frame, t*32+b] f32 buffer (8 steps per copy), converting fp16->f32.
#  - epilogue: PE transposes 128x128 (frame x time) tiles of Y, DVE
#    scalar_tensor_tensor applies the hann window and overlap-adds into ACC;
#    bursts for time-quarters j=0..2 are interleaved into the wavefront.
#  - output: PE transposes ACC back to sample-major, a TT applies 64/norm,
#    contiguous-row DMAs write the cropped result.
import numpy as np

HOP, WIN, PAD = 128, 512, 192
B, T, S = 32, 131072, 11
F = T // HOP           # 1024
NCORE = 8
NB = B // NCORE        # 4 batch rows per core
NFR = NB * F           # 4096 frames per core
NBK = NFR // 128       # 32 frame blocks
LFULL = T + 2 * PAD    # 131456
NCELL = LFULL // HOP   # 1027
ACCW = 1028
NSTEP = WIN + S - 1    # 522 wavefront steps (g = 0..521)
RNG = 16               # ring depth (slabs)
CHUNK = 128            # staging chunk (time samples)
NCH = WIN // CHUNK     # 4
K0S = [1 + 128 * i for i in range(8)] + [898]
FSCL = 1.0 / 64.0      # fp16 pre-scale (folded into gain / norm tables)

# chains: (name, engine attr, nbc blocks, b0, fp16?)
CHAINS = [("a", "vector", 15, 0, True),
          ("b", "vector", 14, 15, True),
          ("p", "gpsimd", 3, 29, False)]

_CACHE = {}


def _hann(n):
    return 0.5 * (1.0 - np.cos(2.0 * np.pi * np.arange(n) / n))


def _build_module():
    import concourse.bass as bass
    import concourse.tile as tile
    from concourse import bacc, mybir
    from concourse.ap import AP

    f32 = mybir.dt.float32
    f16 = mybir.dt.float16
    mult = mybir.AluOpType.mult
    add = mybir.AluOpType.add
    CopyF = mybir.ActivationFunctionType.Copy

    nc = bacc.Bacc("TRN2", target_bir_lowering=False, debug=False)
    ex_in = nc.dram_tensor("ex", [NB, T], f32, kind="ExternalInput").ap()
    ct_in = {}
    for nm, _, nbc, _, fp16 in CHAINS:
        ct_in[nm] = nc.dram_tensor(f"ct_{nm}", [128, 2 * S * nbc],
                                   f16 if fp16 else f32,
                                   kind="ExternalInput").ap()
    gb_in = nc.dram_tensor("gbrep", [128, 8 * NBK], f32, kind="ExternalInput").ap()
    win_in = nc.dram_tensor("win4", [128, 4], f32, kind="ExternalInput").ap()
    rnt_in = nc.dram_tensor("rnt", [128, 9 * 128], f32, kind="ExternalInput").ap()
    id_in = nc.dram_tensor("idn", [128, 128], f32, kind="ExternalInput").ap()
    out = nc.dram_tensor("out", [NB, T], f32, kind="ExternalOutput").ap()
    expd = nc.dram_tensor("expd", [NB, LFULL], f32).ap()

    XSW = CHUNK * NBK  # 4096 cols per staging chunk
    YW = WIN * NBK     # 16384

    with tile.TileContext(nc) as tc:
        with (
            tc.tile_pool(name="state", bufs=1) as st,
            tc.tile_pool(name="scratch", bufs=2) as sp,
            tc.tile_pool(name="xs", bufs=2) as xp,
            tc.tile_pool(name="psum", bufs=4, space="PSUM") as pp,
        ):
            chains = []
            for nm, eng_attr, nbc, b0, fp16 in CHAINS:
                dt = f16 if fp16 else f32
                W2 = (S + 1) * nbc
                ch = dict(
                    nm=nm, eng=getattr(nc, eng_attr), nbc=nbc, b0=b0,
                    dt=dt, W2=W2, Lm=S * nbc,
                    ring=st.tile([128, RNG * W2], dt, name=f"ring{nm}"),
                    C=st.tile([128, 2 * S * nbc], dt, name=f"C{nm}"),
                )
                chains.append(ch)

            Y = st.tile([128, YW], f32, name="Y")
            ACC = st.tile([128, NB * ACCW], f32, name="ACC")
            GBR = st.tile([128, 8 * NBK], f32, name="GBR")
            WIN4 = st.tile([128, 4], f32, name="WIN4")
            RNT = st.tile([128, 9 * 128], f32, name="RNT")
            IDN = st.tile([128, 128], f32, name="IDN")
            ZER = st.tile([128, 6], f32, name="ZER")

            # ---- one-time loads + init ----
            for ch in chains:
                nc.sync.dma_start(ch["C"][:], ct_in[ch["nm"]])
                ch["eng"].memset(ch["ring"][:], 0.0)
            nc.sync.dma_start(GBR[:], gb_in)
            nc.sync.dma_start(WIN4[:], win_in)
            nc.sync.dma_start(RNT[:], rnt_in)
            nc.sync.dma_start(IDN[:], id_in)
            nc.vector.memset(ACC[:], 0.0)
            nc.vector.memset(ZER[:], 0.0)

            # padded excitation in DRAM: expd[:, PAD:PAD+T] = ex, edges 0
            nc.sync.dma_start(
                AP(expd.tensor, PAD, [[LFULL, NB], [1, T]]),
                AP(ex_in.tensor, 0, [[T, NB], [1, T]]))
            nc.sync.dma_start(
                AP(expd.tensor, 0, [[LFULL, NB], [1, PAD]]),
                AP(ZER[:].tensor, 0, [[6, 128], [1, 6]]))
            nc.sync.dma_start(
                AP(expd.tensor, PAD + T, [[LFULL, NB], [1, PAD]]),
                AP(ZER[:].tensor, 0, [[6, 128], [1, 6]]))

            # staging chunk DMA: Xs[p, (t%CHUNK)*32 + b] = expd[beta, f*HOP+t]
            xs_tiles = {}

            def issue_chunk(c):
                xs = xp.tile([128, XSW], f32, tag="xs", name=f"xs{c}")
                xs_tiles[c] = xs
                xst = xs[:].tensor
                for beta in range(NB):
                    nc.sync.dma_start(
                        AP(xst, 8 * beta * CHUNK,
                           [[XSW, 128], [CHUNK, 8], [1, CHUNK]]),
                        AP(expd.tensor, beta * LFULL + CHUNK * c,
                           [[128, 128], [16384, 8], [1, CHUNK]]))

            # staging compute: ring slot-0 [tau0, tau0+8) = Xs * gain
            def stage_batch(t0):
                xs = xs_tiles[t0 // CHUNK]
                xst = xs[:].tensor
                for ch in chains:
                    W2, nbc, b0 = ch["W2"], ch["nbc"], ch["b0"]
                    rt = ch["ring"][:].tensor
                    ch["eng"].tensor_tensor(
                        AP(rt, (t0 % RNG) * W2,
                           [[RNG * W2, 128], [W2, 8], [1, nbc]]),
                        AP(xst, b0 * CHUNK + (t0 % CHUNK),
                           [[XSW, 128], [1, 8], [CHUNK, nbc]]),
                        AP(GBR[:].tensor, b0, [[8 * NBK, 128], [NBK, 8], [1, nbc]]),
                        op=mult)

            # y11 export: Y[p, t*32+b] = ring[pos(t+11), slot 11] (ACT, converts)
            def export_batch(t0, n):
                for ch in chains:
                    W2, nbc, b0 = ch["W2"], ch["nbc"], ch["b0"]
                    rt = ch["ring"][:].tensor
                    t = t0
                    left = n
                    while left > 0:
                        pos = (t + S) % RNG
                        m = min(left, RNG - pos)
                        nc.scalar.activation(
                            AP(Y[:].tensor, t * NBK + b0,
                               [[YW, 128], [NBK, m], [1, nbc]]),
                            AP(rt, pos * W2 + S * nbc,
                               [[RNG * W2, 128], [W2, m], [1, nbc]]),
                            CopyF)
                        t += m
                        left -= m

            # epilogue unit: transpose + window-accumulate for (j, b)
            def epi_op(j, b):
                beta, bb = divmod(b, 8)
                ps = pp.tile([128, 128], f32, tag="ps", name="ps")
                nc.tensor.transpose(
                    ps[:],
                    AP(Y[:].tensor, j * 128 * NBK + b, [[YW, 128], [NBK, 128]]),
                    IDN[:])
                k0 = beta * ACCW + bb * 128 + j
                nc.vector.scalar_tensor_tensor(
                    ACC[:, k0:k0 + 128], ps[:], WIN4[:, j:j + 1],
                    ACC[:, k0:k0 + 128], op0=mult, op1=add)

            # wavefront step for one chain
            def wf_step(ch, g):
                eng, nbc, W2, Lm, dt = ch["eng"], ch["nbc"], ch["W2"], ch["Lm"], ch["dt"]
                rt = ch["ring"][:].tensor
                Ct = ch["C"][:].tensor
                RW = RNG * W2
                s_lo = max(1, g - (WIN - 2))   # g-510
                s_hi = min(S, g + 1)
                L = (s_hi - s_lo + 1) * nbc
                pp_ = (g - 1) % RNG
                pc = g % RNG
                pn = (g + 1) % RNG
                pr = sp.tile([128, 2 * Lm], dt, tag=f"pr{ch['nm']}",
                             name=f"pr{ch['nm']}")
                t2 = sp.tile([128, Lm], dt, tag=f"t2{ch['nm']}",
                             name=f"t2{ch['nm']}")
                prt = pr[:].tensor
                t2t = t2[:].tensor
                co = (s_lo - 1) * nbc
                if pc != 0:
                    eng.tensor_tensor(
                        AP(prt, 0, [[2 * Lm, 128], [Lm, 2], [1, L]]),
                        AP(rt, pp_ * W2 + s_lo * nbc, [[RW, 128], [W2, 2], [1, L]]),
                        AP(Ct, co, [[2 * Lm, 128], [Lm, 2], [1, L]]),
                        op=mult)
                else:  # ring wrap between slabs g-1 (pos 15) and g (pos 0)
                    eng.tensor_tensor(
                        AP(prt, 0, [[2 * Lm, 128], [1, L]]),
                        AP(rt, pp_ * W2 + s_lo * nbc, [[RW, 128], [1, L]]),
                        AP(Ct, co, [[2 * Lm, 128], [1, L]]),
                        op=mult)
                    eng.tensor_tensor(
                        AP(prt, Lm, [[2 * Lm, 128], [1, L]]),
                        AP(rt, pc * W2 + s_lo * nbc, [[RW, 128], [1, L]]),
                        AP(Ct, Lm + co, [[2 * Lm, 128], [1, L]]),
                        op=mult)
                yield
                eng.tensor_tensor(
                    AP(t2t, 0, [[Lm, 128], [1, L]]),
                    AP(prt, 0, [[2 * Lm, 128], [1, L]]),
                    AP(prt, Lm, [[2 * Lm, 128], [1, L]]),
                    op=add)
                yield
                eng.tensor_tensor(
                    AP(rt, pn * W2 + s_lo * nbc, [[RW, 128], [1, L]]),
                    AP(t2t, 0, [[Lm, 128], [1, L]]),
                    AP(rt, pc * W2 + (s_lo - 1) * nbc, [[RW, 128], [1, L]]),
                    op=add)
                yield

            # ---- preamble staging ----
            issue_chunk(0)
            issue_chunk(1)
            stage_batch(0)
            stage_batch(8)

            # epilogue burst schedule: (g -> op) for j = 0..2
            epi_sched = {}
            for j in range(3):
                g0 = 128 * (j + 1) + 16
                for i, b in enumerate(range(NBK)):
                    epi_sched[g0 + 2 * i] = (j, b)

            # ---- wavefront ----
            for g in range(NSTEP):
                if g in (110, 238):
                    issue_chunk(2 if g == 110 else 3)
                if (g + 12) % 8 == 0 and g + 12 < WIN:
                    stage_batch(g + 12)
                if g >= 19 and (g - 19) % 8 == 0:
                    export_batch(g - 19, 8)
                if g in epi_sched:
                    epi_op(*epi_sched[g])
                steps = [wf_step(ch, g) for ch in chains]
                for _ in range(3):
                    for it in steps:
                        next(it, None)

            # ---- drain: remaining exports, epilogue j=3, output ----
            export_batch(504, 8)
            for b in range(NBK):
                epi_op(3, b)

            at = ACC[:].tensor
            for beta in range(NB):
                for i, k0 in enumerate(K0S):
                    ps = pp.tile([128, 128], f32, tag="pso", name="pso")
                    nc.tensor.transpose(
                        ps[:], ACC[:, beta * ACCW + k0:beta * ACCW + k0 + 128],
                        IDN[:])
                    ot = sp.tile([128, 128], f32, tag="ot", name="ot")
                    nc.vector.tensor_tensor(ot[:], ps[:],
                                            RNT[:, i * 128:(i + 1) * 128], op=mult)
                    o_t = ot[:].tensor
                    if i == 0:
                        nc.sync.dma_start(
                            AP(out.tensor, beta * T, [[1, 1], [1, 64]]),
                            AP(o_t, 64, [[128, 1], [1, 64]]))
                        nc.sync.dma_start(
                            AP(out.tensor, beta * T + 64, [[128, 127], [1, 128]]),
                            AP(o_t, 128, [[128, 127], [1, 128]]))
                    elif i < 8:
                        nc.sync.dma_start(
                            AP(out.tensor, beta * T + k0 * 128 - PAD,
                               [[128, 128], [1, 128]]),
                            AP(o_t, 0, [[128, 128], [1, 128]]))
                    else:
                        nc.sync.dma_start(
                            AP(out.tensor, beta * T + 1025 * 128 - PAD,
                               [[1, 1], [1, 64]]),
                            AP(o_t, 127 * 128, [[128, 1], [1, 64]]))

    nc.compile()
    return nc


def _host_prep(ex, gain, biquads):
    # per-core host tables; frame n = beta*F + f -> p = n % 128, b = n // 128
    f32 = np.float32
    a0 = biquads[..., 0].astype(f32)
    a1 = biquads[..., 1].astype(f32)
    a2 = biquads[..., 2].astype(f32)
    c1 = (-a1 / a0).astype(f32)          # [NB, F, S]
    c2 = (-a2 / a0).astype(f32)
    gain_eff = (gain.astype(f32) * np.prod((1.0 / a0).astype(f32), axis=-1)
                * f32(FSCL)).astype(f32)

    # [p, s, b]
    c1pb = c1.reshape(NBK, 128, S).transpose(1, 2, 0)
    c2pb = c2.reshape(NBK, 128, S).transpose(1, 2, 0)
    cts = {}
    for nm, _, nbc, b0, fp16 in CHAINS:
        t = np.concatenate(
            [c2pb[:, :, b0:b0 + nbc].reshape(128, S * nbc),
             c1pb[:, :, b0:b0 + nbc].reshape(128, S * nbc)], axis=1)
        cts[f"ct_{nm}"] = np.ascontiguousarray(
            t.astype(np.float16 if fp16 else f32))
    g = gain_eff.reshape(NBK, 128).T     # [p, b]
    cts["gbrep"] = np.ascontiguousarray(np.tile(g, (1, 8)))  # [p, r*32+b]
    return cts


def _host_consts():
    f32 = np.float32
    win = _hann(WIN).astype(f32)
    WIN4 = np.ascontiguousarray(win.reshape(4, 128).T)
    norm = np.zeros(LFULL, f32)
    idx = (np.arange(F)[:, None] * HOP + np.arange(WIN)[None, :]).reshape(-1)
    np.add.at(norm, idx, np.broadcast_to(win, (F, WIN)).reshape(-1))
    rn = np.zeros_like(norm)
    nz = norm != 0
    rn[nz] = (f32(1.0 / FSCL) / norm[nz]).astype(f32)  # un-scale fp16 here
    rn2 = rn.reshape(NCELL, 128)
    RNT = np.zeros((128, 9 * 128), f32)
    for i, k0 in enumerate(K0S):
        RNT[:, i * 128:(i + 1) * 128] = rn2[k0:k0 + 128, :]
    IDN = np.eye(128, dtype=f32)
    return WIN4, RNT, IDN


def make_in_maps(ex, gain, biquads):
    ex = np.asarray(ex, np.float32)
    gain = np.asarray(gain, np.float32)
    biquads = np.asarray(biquads, np.float32)
    WIN4, RNT, IDN = _host_consts()
    in_maps = []
    for ci in range(NCORE):
        sl = slice(ci * NB, (ci + 1) * NB)
        m = {"ex": np.ascontiguousarray(ex[sl]),
             "win4": WIN4, "rnt": RNT, "idn": IDN}
        m.update(_host_prep(ex[sl], gain[sl], biquads[sl]))
        in_maps.append(m)
    return in_maps


def kernel(ex, gain, biquads):
    from concourse.bass_utils import run_bass_kernel_spmd

    if "nc" not in _CACHE:
        _CACHE["nc"] = _build_module()
    nc = _CACHE["nc"]
    in_maps = make_in_maps(ex, gain, biquads)
    res = run_bass_kernel_spmd(nc, in_maps, list(range(NCORE)))
    out = np.concatenate([res.results[ci]["out"] for ci in range(NCORE)], axis=0)
    return out.astype(np.float32)


# revision 6
# speedup vs baseline: 1.3047x; 1.1956x over previous
# Trainium2 Bass kernel: batched second-order LPC synthesis
# (frame unfold -> gain -> 11 cascaded biquads -> hann window -> overlap-add -> norm)
#
# Sharding: pure data parallel over batch. 32 batch rows / 8 cores = 4 rows per
# core = 4096 frames, laid out as 128 partitions x 32 frame-blocks.
#
# Device algorithm (v2, "slab ring"):
#  - the 11-section cascade runs as a wavefront over (section, time), but the
#    state is stored in per-step SLABS indexed by tau = t + s - 1: at step g
#    every active section writes time t = g-s+1, and ALL those cells share
#    tau = g+1. A slab is [slot s=0..11][block b] contiguous, so every
#    wavefront operand is a stride-1 run: slot 0 holds the gained input x[t],
#    slots 1..11 hold section outputs.
#  - per step, three ops per chain: pr = (y2-run|y1-run) * (c2-run|c1-run)
#    (one TT over both slabs g-1,g), t2 = pr_lo + pr_hi, y-run = t2 + u-run
#    (u = previous section's output = same slab shifted one slot down).
#  - slabs live in a 16-deep ring (lifetime of a slab is 2 steps).
#  - the DVE state is fp16 (inputs pre-scaled by 1/64; un-scaled in the output
#    norm table): packed stride-1 APs + 2-byte dtype put DVE TT in 2x mode.
#    Two interleaved DVE chains (15+14 blocks) hide the ~95ns semaphore
#    latency between dependent ops. gpsimd (Pool) runs 3 blocks in f32 as an
#    independent third chain.
#  - the scalar (ACT) engine exports section-11 outputs from the ring to a
#    [frame, t*32+b] f32 buffer (8 steps per copy), converting fp16->f32.
#  - epilogue: PE transposes 128x128 (frame x time) tiles of Y, DVE
#    scalar_tensor_tensor applies the hann window and overlap-adds into ACC;
#    bursts for time-quarters j=0..2 are interleaved into the wavefront.
#  - output: PE transposes ACC back to sample-major, a TT applies 64/norm,
#    contiguous-row DMAs write the cropped result.
import numpy as np

HOP, WIN, PAD = 128, 512, 192
B, T, S = 32, 131072, 11
F = T // HOP           # 1024
NCORE = 8
NB = B // NCORE        # 4 batch rows per core
NFR = NB * F           # 4096 frames per core
NBK = NFR // 128       # 32 frame blocks
LFULL = T + 2 * PAD    # 131456
NCELL = LFULL // HOP   # 1027
ACCW = 1028
NSTEP = WIN + S - 1    # 522 wavefront steps (g = 0..521)
RNG = 16               # ring depth (slabs)
CHUNK = 128            # staging chunk (time samples)
NCH = WIN // CHUNK     # 4
K0S = [1 + 128 * i for i in range(8)] + [898]
FSCL = 1.0 / 64.0      # fp16 pre-scale (folded into gain / norm tables)

# chains: (name, engine attr, nbc blocks, b0, fp16?)
CHAINS = [("a", "vector", 15, 0, True),
          ("b", "vector", 14, 15, True),
          ("p", "gpsimd", 3, 29, False)]

_CACHE = {}


def _hann(n):
    return 0.5 * (1.0 - np.cos(2.0 * np.pi * np.arange(n) / n))


def _build_module():
    import concourse.bass as bass
    import concourse.tile as tile
    from concourse import bacc, mybir
    from concourse.ap import AP

    f32 = mybir.dt.float32
    f16 = mybir.dt.float16
    mult = mybir.AluOpType.mult
    add = mybir.AluOpType.add
    CopyF = mybir.ActivationFunctionType.Copy

    nc = bacc.Bacc("TRN2", target_bir_lowering=False, debug=False)
    ex_in = nc.dram_tensor("ex", [NB, T], f32, kind="ExternalInput").ap()
    ct_in = {}
    for nm, _, nbc, _, fp16 in CHAINS:
        ct_in[nm] = nc.dram_tensor(f"ct_{nm}", [128, 2 * S * nbc],
                                   f16 if fp16 else f32,
                                   kind="ExternalInput").ap()
    gb_in = nc.dram_tensor("gbrep", [128, 8 * NBK], f32, kind="ExternalInput").ap()
    win_in = nc.dram_tensor("win4", [128, 4], f32, kind="ExternalInput").ap()
    rnt_in = nc.dram_tensor("rnt", [128, 9 * 128], f32, kind="ExternalInput").ap()
    id_in = nc.dram_tensor("idn", [128, 128], f32, kind="ExternalInput").ap()
    out = nc.dram_tensor("out", [NB, T], f32, kind="ExternalOutput").ap()
    expd = nc.dram_tensor("expd", [NB, LFULL], f32).ap()

    XSW = CHUNK * NBK  # 4096 cols per staging chunk
    YW = WIN * NBK     # 16384

    with tile.TileContext(nc) as tc:
        with (
            tc.tile_pool(name="state", bufs=1) as st,
            tc.tile_pool(name="scratch", bufs=2) as sp,
            tc.tile_pool(name="xs", bufs=2) as xp,
            tc.tile_pool(name="psum", bufs=4, space="PSUM") as pp,
        ):
            # ring layout is SLOT-SEGREGATED: col = s*(RNG*nbc) + pos*nbc + b.
            # The tile dep tracker works on address intervals, so each op
            # class (staging: slot 0, wavefront: slots 1-11, export: slot 11)
            # stays in a narrow interval and false cross-engine deps vanish.
            chains = []
            for nm, eng_attr, nbc, b0, fp16 in CHAINS:
                dt = f16 if fp16 else f32
                ch = dict(
                    nm=nm, eng=getattr(nc, eng_attr), nbc=nbc, b0=b0,
                    dt=dt, SR=RNG * nbc, Lm=S * nbc,
                    ring=st.tile([128, (S + 1) * RNG * nbc], dt, name=f"ring{nm}"),
                    C=st.tile([128, 2 * S * nbc], dt, name=f"C{nm}"),
                    Y=st.tile([128, WIN * nbc], f32, name=f"Y{nm}"),
                )
                chains.append(ch)
            ACC = st.tile([128, NB * ACCW], f32, name="ACC")
            GBR = st.tile([128, 8 * NBK], f32, name="GBR")
            WIN4 = st.tile([128, 4], f32, name="WIN4")
            RNT = st.tile([128, 9 * 128], f32, name="RNT")
            IDN = st.tile([128, 128], f32, name="IDN")
            ZER = st.tile([128, 6], f32, name="ZER")

            # ---- one-time loads + init ----
            for ch in chains:
                nc.sync.dma_start(ch["C"][:], ct_in[ch["nm"]])
                ch["eng"].memset(ch["ring"][:], 0.0)
            nc.sync.dma_start(GBR[:], gb_in)
            nc.sync.dma_start(WIN4[:], win_in)
            nc.sync.dma_start(RNT[:], rnt_in)
            nc.sync.dma_start(IDN[:], id_in)
            nc.vector.memset(ACC[:], 0.0)
            nc.vector.memset(ZER[:], 0.0)

            # padded excitation in DRAM: expd[:, PAD:PAD+T] = ex, edges 0
            nc.sync.dma_start(
                AP(expd.tensor, PAD, [[LFULL, NB], [1, T]]),
                AP(ex_in.tensor, 0, [[T, NB], [1, T]]))
            nc.sync.dma_start(
                AP(expd.tensor, 0, [[LFULL, NB], [1, PAD]]),
                AP(ZER[:].tensor, 0, [[6, 128], [1, 6]]))
            nc.sync.dma_start(
                AP(expd.tensor, PAD + T, [[LFULL, NB], [1, PAD]]),
                AP(ZER[:].tensor, 0, [[6, 128], [1, 6]]))

            # staging chunk DMA: Xs[p, (t%CHUNK)*32 + b] = expd[beta, f*HOP+t]
            xs_tiles = {}

            def issue_chunk(c):
                xs = xp.tile([128, XSW], f32, tag="xs", name=f"xs{c}")
                xs_tiles[c] = xs
                xst = xs[:].tensor
                for beta in range(NB):
                    nc.sync.dma_start(
                        AP(xst, 8 * beta * CHUNK,
                           [[XSW, 128], [CHUNK, 8], [1, CHUNK]]),
                        AP(expd.tensor, beta * LFULL + CHUNK * c,
                           [[128, 128], [16384, 8], [1, CHUNK]]))

            # staging compute: ring slot-0 positions [tau0, tau0+8) = Xs * gain
            def stage_batch(t0):
                xs = xs_tiles[t0 // CHUNK]
                xst = xs[:].tensor
                for ch in chains:
                    nbc, b0 = ch["nbc"], ch["b0"]
                    rt = ch["ring"][:].tensor
                    RWT = (S + 1) * RNG * nbc
                    ch["eng"].tensor_tensor(
                        AP(rt, (t0 % RNG) * nbc, [[RWT, 128], [1, 8 * nbc]]),
                        AP(xst, b0 * CHUNK + (t0 % CHUNK),
                           [[XSW, 128], [1, 8], [CHUNK, nbc]]),
                        AP(GBR[:].tensor, b0, [[8 * NBK, 128], [NBK, 8], [1, nbc]]),
                        op=mult)

            # y11 export: Ych[p, t*nbc+b] = ring[slot 11, pos(t+11)]
            # (ACT for the fp16 DVE chains — converts to f32; Pool exports its
            # own f32 blocks so the ACT queue never waits on Pool)
            def export_batch(t0, n):
                for ch in chains:
                    nbc = ch["nbc"]
                    rt = ch["ring"][:].tensor
                    RWT = (S + 1) * RNG * nbc
                    YWc = WIN * nbc
                    t = t0
                    left = n
                    while left > 0:
                        pos = (t + S) % RNG
                        m = min(left, RNG - pos)
                        src = AP(rt, S * ch["SR"] + pos * nbc,
                                 [[RWT, 128], [1, m * nbc]])
                        dst = AP(ch["Y"][:].tensor, t * nbc,
                                 [[YWc, 128], [1, m * nbc]])
                        if ch["nm"] == "p":
                            nc.gpsimd.tensor_copy(dst, src)
                        else:
                            nc.scalar.activation(dst, src, CopyF)
                        t += m
                        left -= m

            # epilogue unit: transpose + window-accumulate for (j, b)
            def epi_op(j, b):
                for ch in chains:
                    if ch["b0"] <= b < ch["b0"] + ch["nbc"]:
                        break
                nbc = ch["nbc"]
                bl = b - ch["b0"]
                beta, bb = divmod(b, 8)
                ps = pp.tile([128, 128], f32, tag="ps", name="ps")
                nc.tensor.transpose(
                    ps[:],
                    AP(ch["Y"][:].tensor, j * 128 * nbc + bl,
                       [[WIN * nbc, 128], [nbc, 128]]),
                    IDN[:])
                k0 = beta * ACCW + bb * 128 + j
                nc.vector.scalar_tensor_tensor(
                    ACC[:, k0:k0 + 128], ps[:], WIN4[:, j:j + 1],
                    ACC[:, k0:k0 + 128], op0=mult, op1=add)

            # wavefront step for one chain
            def wf_step(ch, g):
                eng, nbc, SR, Lm = ch["eng"], ch["nbc"], ch["SR"], ch["Lm"]
                rt = ch["ring"][:].tensor
                Ct = ch["C"][:].tensor
                RWT = (S + 1) * RNG * nbc
                s_lo = max(1, g - (WIN - 2))   # g-510
                s_hi = min(S, g + 1)
                ns = s_hi - s_lo + 1
                L = ns * nbc
                pp_ = (g - 1) % RNG
                pc = g % RNG
                pn = (g + 1) % RNG
                pr = sp.tile([128, 2 * Lm], ch["dt"], tag=f"pr{ch['nm']}",
                             name=f"pr{ch['nm']}")
                t2 = sp.tile([128, Lm], ch["dt"], tag=f"t2{ch['nm']}",
                             name=f"t2{ch['nm']}")
                prt = pr[:].tensor
                t2t = t2[:].tensor
                co = (s_lo - 1) * nbc
                if pc != 0:
                    eng.tensor_tensor(
                        AP(prt, 0, [[2 * Lm, 128], [Lm, 2], [nbc, ns], [1, nbc]]),
                        AP(rt, s_lo * SR + pp_ * nbc,
                           [[RWT, 128], [nbc, 2], [SR, ns], [1, nbc]]),
                        AP(Ct, co, [[2 * Lm, 128], [Lm, 2], [nbc, ns], [1, nbc]]),
                        op=mult)
                else:  # ring wrap between slabs g-1 (pos 15) and g (pos 0)
                    eng.tensor_tensor(
                        AP(prt, 0, [[2 * Lm, 128], [nbc, ns], [1, nbc]]),
                        AP(rt, s_lo * SR + pp_ * nbc,
                           [[RWT, 128], [SR, ns], [1, nbc]]),
                        AP(Ct, co, [[2 * Lm, 128], [1, L]]),
                        op=mult)
                    eng.tensor_tensor(
                        AP(prt, Lm, [[2 * Lm, 128], [nbc, ns], [1, nbc]]),
                        AP(rt, s_lo * SR + pc * nbc,
                           [[RWT, 128], [SR, ns], [1, nbc]]),
                        AP(Ct, Lm + co, [[2 * Lm, 128], [1, L]]),
                        op=mult)
                yield
                eng.tensor_tensor(
                    AP(t2t, 0, [[Lm, 128], [1, L]]),
                    AP(prt, 0, [[2 * Lm, 128], [1, L]]),
                    AP(prt, Lm, [[2 * Lm, 128], [1, L]]),
                    op=add)
                yield
                eng.tensor_tensor(
                    AP(rt, s_lo * SR + pn * nbc,
                       [[RWT, 128], [SR, ns], [1, nbc]]),
                    AP(t2t, 0, [[Lm, 128], [1, L]]),
                    AP(rt, (s_lo - 1) * SR + pc * nbc,
                       [[RWT, 128], [SR, ns], [1, nbc]]),
                    op=add)
                yield

            # ---- preamble staging ----
            issue_chunk(0)
            issue_chunk(1)
            stage_batch(0)
            stage_batch(8)

            # epilogue burst schedule: (g -> op) for j = 0..2
            epi_sched = {}
            for j in range(3):
                g0 = 128 * (j + 1) + 16
                for i, b in enumerate(range(NBK)):
                    epi_sched[g0 + 2 * i] = (j, b)

            # ---- wavefront ----
            for g in range(NSTEP):
                if g in (110, 238):
                    issue_chunk(2 if g == 110 else 3)
                if (g + 12) % 8 == 0 and g + 12 < WIN:
                    stage_batch(g + 12)
                if g >= 19 and (g - 19) % 8 == 0:
                    export_batch(g - 19, 8)
                if g in epi_sched:
                    epi_op(*epi_sched[g])
                steps = [wf_step(ch, g) for ch in chains]
                for _ in range(3):
                    for it in steps:
                        next(it, None)

            # ---- drain: remaining exports, epilogue j=3, output ----
            export_batch(504, 8)
            for b in range(NBK):
                epi_op(3, b)

            at = ACC[:].tensor
            for beta in range(NB):
                for i, k0 in enumerate(K0S):
                    ps = pp.tile([128, 128], f32, tag="pso", name="pso")
                    nc.tensor.transpose(
                        ps[:], ACC[:, beta * ACCW + k0:beta * ACCW + k0 + 128],
                        IDN[:])
                    ot = sp.tile([128, 128], f32, tag="ot", name="ot")
                    nc.vector.tensor_tensor(ot[:], ps[:],
                                            RNT[:, i * 128:(i + 1) * 128], op=mult)
                    o_t = ot[:].tensor
                    if i == 0:
                        nc.sync.dma_start(
                            AP(out.tensor, beta * T, [[1, 1], [1, 64]]),
                            AP(o_t, 64, [[128, 1], [1, 64]]))
                        nc.sync.dma_start(
                            AP(out.tensor, beta * T + 64, [[128, 127], [1, 128]]),
                            AP(o_t, 128, [[128, 127], [1, 128]]))
                    elif i < 8:
                        nc.sync.dma_start(
                            AP(out.tensor, beta * T + k0 * 128 - PAD,
                               [[128, 128], [1, 128]]),
                            AP(o_t, 0, [[128, 128], [1, 128]]))
                    else:
                        nc.sync.dma_start(
                            AP(out.tensor, beta * T + 1025 * 128 - PAD,
                               [[1, 1], [1, 64]]),
                            AP(o_t, 127 * 128, [[128, 1], [1, 64]]))

    nc.compile()
    return nc


def _host_prep(ex, gain, biquads):
    # per-core host tables; frame n = beta*F + f -> p = n % 128, b = n // 128
    f32 = np.float32
    a0 = biquads[..., 0].astype(f32)
    a1 = biquads[..., 1].astype(f32)
    a2 = biquads[..., 2].astype(f32)
    c1 = (-a1 / a0).astype(f32)          # [NB, F, S]
    c2 = (-a2 / a0).astype(f32)
    gain_eff = (gain.astype(f32) * np.prod((1.0 / a0).astype(f32), axis=-1)
                * f32(FSCL)).astype(f32)

    # [p, s, b]
    c1pb = c1.reshape(NBK, 128, S).transpose(1, 2, 0)
    c2pb = c2.reshape(NBK, 128, S).transpose(1, 2, 0)
    cts = {}
    for nm, _, nbc, b0, fp16 in CHAINS:
        t = np.concatenate(
            [c2pb[:, :, b0:b0 + nbc].reshape(128, S * nbc),
             c1pb[:, :, b0:b0 + nbc].reshape(128, S * nbc)], axis=1)
        cts[f"ct_{nm}"] = np.ascontiguousarray(
            t.astype(np.float16 if fp16 else f32))
    g = gain_eff.reshape(NBK, 128).T     # [p, b]
    cts["gbrep"] = np.ascontiguousarray(np.tile(g, (1, 8)))  # [p, r*32+b]
    return cts


def _host_consts():
    f32 = np.float32
    win = _hann(WIN).astype(f32)
    WIN4 = np.ascontiguousarray(win.reshape(4, 128).T)
    norm = np.zeros(LFULL, f32)
    idx = (np.arange(F)[:, None] * HOP + np.arange(WIN)[None, :]).reshape(-1)
    np.add.at(norm, idx, np.broadcast_to(win, (F, WIN)).reshape(-1))
    rn = np.zeros_like(norm)
    nz = norm != 0
    rn[nz] = (f32(1.0 / FSCL) / norm[nz]).astype(f32)  # un-scale fp16 here
    rn2 = rn.reshape(NCELL, 128)
    RNT = np.zeros((128, 9 * 128), f32)
    for i, k0 in enumerate(K0S):
        RNT[:, i * 128:(i + 1) * 128] = rn2[k0:k0 + 128, :]
    IDN = np.eye(128, dtype=f32)
    return WIN4, RNT, IDN


def make_in_maps(ex, gain, biquads):
    ex = np.asarray(ex, np.float32)
    gain = np.asarray(gain, np.float32)
    biquads = np.asarray(biquads, np.float32)
    WIN4, RNT, IDN = _host_consts()
    in_maps = []
    for ci in range(NCORE):
        sl = slice(ci * NB, (ci + 1) * NB)
        m = {"ex": np.ascontiguousarray(ex[sl]),
             "win4": WIN4, "rnt": RNT, "idn": IDN}
        m.update(_host_prep(ex[sl], gain[sl], biquads[sl]))
        in_maps.append(m)
    return in_maps


def kernel(ex, gain, biquads):
    from concourse.bass_utils import run_bass_kernel_spmd

    if "nc" not in _CACHE:
        _CACHE["nc"] = _build_module()
    nc = _CACHE["nc"]
    in_maps = make_in_maps(ex, gain, biquads)
    res = run_bass_kernel_spmd(nc, in_maps, list(range(NCORE)))
    out = np.concatenate([res.results[ci]["out"] for ci in range(NCORE)], axis=0)
    return out.astype(np.float32)


# revision 9
# speedup vs baseline: 1.3396x; 1.0268x over previous
# Trainium2 Bass kernel: batched second-order LPC synthesis
# (frame unfold -> gain -> 11 cascaded biquads -> hann window -> overlap-add -> norm)
#
# Sharding: pure data parallel over batch. 32 batch rows / 8 cores = 4 rows per
# core = 4096 frames, laid out as 128 partitions x 32 frame-blocks.
#
# Device algorithm (v2, "slab ring"):
#  - the 11-section cascade runs as a wavefront over (section, time), but the
#    state is stored in per-step SLABS indexed by tau = t + s - 1: at step g
#    every active section writes time t = g-s+1, and ALL those cells share
#    tau = g+1. A slab is [slot s=0..11][block b] contiguous, so every
#    wavefront operand is a stride-1 run: slot 0 holds the gained input x[t],
#    slots 1..11 hold section outputs.
#  - per step, three ops per chain: pr = (y2-run|y1-run) * (c2-run|c1-run)
#    (one TT over both slabs g-1,g), t2 = pr_lo + pr_hi, y-run = t2 + u-run
#    (u = previous section's output = same slab shifted one slot down).
#  - slabs live in a 16-deep ring (lifetime of a slab is 2 steps).
#  - the DVE state is fp16 (inputs pre-scaled by 1/64; un-scaled in the output
#    norm table): packed stride-1 APs + 2-byte dtype put DVE TT in 2x mode.
#    Two interleaved DVE chains (15+14 blocks) hide the ~95ns semaphore
#    latency between dependent ops. gpsimd (Pool) runs 3 blocks in f32 as an
#    independent third chain.
#  - the scalar (ACT) engine exports section-11 outputs from the ring to a
#    [frame, t*32+b] f32 buffer (8 steps per copy), converting fp16->f32.
#  - epilogue: PE transposes 128x128 (frame x time) tiles of Y, DVE
#    scalar_tensor_tensor applies the hann window and overlap-adds into ACC;
#    bursts for time-quarters j=0..2 are interleaved into the wavefront.
#  - output: PE transposes ACC back to sample-major, a TT applies 64/norm,
#    contiguous-row DMAs write the cropped result.
import numpy as np

HOP, WIN, PAD = 128, 512, 192
B, T, S = 32, 131072, 11
F = T // HOP           # 1024
NCORE = 8
NB = B // NCORE        # 4 batch rows per core
NFR = NB * F           # 4096 frames per core
NBK = NFR // 128       # 32 frame blocks
LFULL = T + 2 * PAD    # 131456
NCELL = LFULL // HOP   # 1027
ACCW = 1028
NSTEP = WIN + S - 1    # 522 wavefront steps (g = 0..521)
RNG = 16               # ring depth (slabs)
CHUNK = 128            # staging chunk (time samples)
NCH = WIN // CHUNK     # 4
K0S = [1 + 128 * i for i in range(8)] + [898]
FSCL = 1.0 / 64.0      # fp16 pre-scale (folded into gain / norm tables)

# chains: (name, engine attr, nbc blocks, b0, fp16?)
CHAINS = [("a", "vector", 14, 0, True),
          ("b", "vector", 14, 14, True),
          ("p", "gpsimd", 4, 28, False)]

_CACHE = {}


def _hann(n):
    return 0.5 * (1.0 - np.cos(2.0 * np.pi * np.arange(n) / n))


def _build_module():
    import concourse.bass as bass
    import concourse.tile as tile
    from concourse import bacc, mybir
    from concourse.ap import AP

    f32 = mybir.dt.float32
    f16 = mybir.dt.float16
    mult = mybir.AluOpType.mult
    add = mybir.AluOpType.add
    CopyF = mybir.ActivationFunctionType.Copy

    nc = bacc.Bacc("TRN2", target_bir_lowering=False, debug=False)
    ex_in = nc.dram_tensor("ex", [NB, T], f32, kind="ExternalInput").ap()
    ct_in = {}
    for nm, _, nbc, _, fp16 in CHAINS:
        ct_in[nm] = nc.dram_tensor(f"ct_{nm}", [128, 2 * S * nbc],
                                   f16 if fp16 else f32,
                                   kind="ExternalInput").ap()
    gb_in = nc.dram_tensor("gbrep", [128, 8 * NBK], f32, kind="ExternalInput").ap()
    win_in = nc.dram_tensor("win4", [128, 4], f32, kind="ExternalInput").ap()
    rnt_in = nc.dram_tensor("rnt", [128, 9 * 128], f32, kind="ExternalInput").ap()
    id_in = nc.dram_tensor("idn", [128, 128], f32, kind="ExternalInput").ap()
    out = nc.dram_tensor("out", [NB, T], f32, kind="ExternalOutput").ap()
    expd = nc.dram_tensor("expd", [NB, LFULL], f32).ap()

    XSW = CHUNK * NBK  # 4096 cols per staging chunk
    YW = WIN * NBK     # 16384

    with tile.TileContext(nc) as tc:
        with (
            tc.tile_pool(name="state", bufs=1) as st,
            tc.tile_pool(name="scratch", bufs=2) as sp,
            tc.tile_pool(name="xs", bufs=2) as xp,
            tc.tile_pool(name="psum", bufs=4, space="PSUM") as pp,
        ):
            # ring layout is SLOT-SEGREGATED: col = s*(RNG*nbc) + pos*nbc + b.
            # The tile dep tracker works on address intervals, so each op
            # class (staging: slot 0, wavefront: slots 1-11, export: slot 11)
            # stays in a narrow interval and false cross-engine deps vanish.
            chains = []
            for nm, eng_attr, nbc, b0, fp16 in CHAINS:
                dt = f16 if fp16 else f32
                ch = dict(
                    nm=nm, eng=getattr(nc, eng_attr), nbc=nbc, b0=b0,
                    dt=dt, SR=RNG * nbc, Lm=S * nbc,
                    ring=st.tile([128, (S + 1) * RNG * nbc], dt, name=f"ring{nm}"),
                    C=st.tile([128, 2 * S * nbc], dt, name=f"C{nm}"),
                    Y=st.tile([128, WIN * nbc], f32, name=f"Y{nm}"),
                )
                chains.append(ch)
            ACC = st.tile([128, NB * ACCW], f32, name="ACC")
            GBR = st.tile([128, 8 * NBK], f32, name="GBR")
            WIN4 = st.tile([128, 4], f32, name="WIN4")
            RNT = st.tile([128, 9 * 128], f32, name="RNT")
            IDN = st.tile([128, 128], f32, name="IDN")
            ZER = st.tile([128, 6], f32, name="ZER")

            # ---- one-time loads + init ----
            for ch in chains:
                nc.sync.dma_start(ch["C"][:], ct_in[ch["nm"]])
                ch["eng"].memset(ch["ring"][:], 0.0)
            nc.sync.dma_start(GBR[:], gb_in)
            nc.sync.dma_start(WIN4[:], win_in)
            nc.sync.dma_start(RNT[:], rnt_in)
            nc.sync.dma_start(IDN[:], id_in)
            nc.vector.memset(ACC[:], 0.0)
            nc.vector.memset(ZER[:], 0.0)

            # padded excitation in DRAM: expd[:, PAD:PAD+T] = ex, edges 0
            nc.sync.dma_start(
                AP(expd.tensor, PAD, [[LFULL, NB], [1, T]]),
                AP(ex_in.tensor, 0, [[T, NB], [1, T]]))
            nc.sync.dma_start(
                AP(expd.tensor, 0, [[LFULL, NB], [1, PAD]]),
                AP(ZER[:].tensor, 0, [[6, 128], [1, 6]]))
            nc.sync.dma_start(
                AP(expd.tensor, PAD + T, [[LFULL, NB], [1, PAD]]),
                AP(ZER[:].tensor, 0, [[6, 128], [1, 6]]))

            # staging chunk DMA: Xs[p, (t%CHUNK)*32 + b] = expd[beta, f*HOP+t]
            xs_tiles = {}

            def issue_chunk(c):
                xs = xp.tile([128, XSW], f32, tag="xs", name=f"xs{c}")
                xs_tiles[c] = xs
                xst = xs[:].tensor
                for beta in range(NB):
                    nc.sync.dma_start(
                        AP(xst, 8 * beta * CHUNK,
                           [[XSW, 128], [CHUNK, 8], [1, CHUNK]]),
                        AP(expd.tensor, beta * LFULL + CHUNK * c,
                           [[128, 128], [16384, 8], [1, CHUNK]]))

            # staging compute: ring slot-0 positions [tau0, tau0+8) = Xs * gain
            def stage_batch(t0):
                xs = xs_tiles[t0 // CHUNK]
                xst = xs[:].tensor
                for ch in chains:
                    nbc, b0 = ch["nbc"], ch["b0"]
                    rt = ch["ring"][:].tensor
                    RWT = (S + 1) * RNG * nbc
                    ch["eng"].tensor_tensor(
                        AP(rt, (t0 % RNG) * nbc, [[RWT, 128], [1, 8 * nbc]]),
                        AP(xst, b0 * CHUNK + (t0 % CHUNK),
                           [[XSW, 128], [1, 8], [CHUNK, nbc]]),
                        AP(GBR[:].tensor, b0, [[8 * NBK, 128], [NBK, 8], [1, nbc]]),
                        op=mult)

            # y11 export: Ych[p, t*nbc+b] = ring[slot 11, pos(t+11)]
            # (ACT for the fp16 DVE chains — converts to f32; Pool exports its
            # own f32 blocks so the ACT queue never waits on Pool)
            def export_batch(t0, n):
                for ch in chains:
                    nbc = ch["nbc"]
                    rt = ch["ring"][:].tensor
                    RWT = (S + 1) * RNG * nbc
                    YWc = WIN * nbc
                    t = t0
                    left = n
                    while left > 0:
                        pos = (t + S) % RNG
                        m = min(left, RNG - pos)
                        src = AP(rt, S * ch["SR"] + pos * nbc,
                                 [[RWT, 128], [1, m * nbc]])
                        dst = AP(ch["Y"][:].tensor, t * nbc,
                                 [[YWc, 128], [1, m * nbc]])
                        if ch["nm"] == "p":
                            nc.gpsimd.tensor_copy(dst, src)
                        else:
                            nc.scalar.activation(dst, src, CopyF)
                        t += m
                        left -= m

            # epilogue unit: transpose + window-accumulate for (j, b)
            def epi_op(j, b):
                for ch in chains:
                    if ch["b0"] <= b < ch["b0"] + ch["nbc"]:
                        break
                nbc = ch["nbc"]
                bl = b - ch["b0"]
                beta, bb = divmod(b, 8)
                ps = pp.tile([128, 128], f32, tag="ps", name="ps")
                nc.tensor.transpose(
                    ps[:],
                    AP(ch["Y"][:].tensor, j * 128 * nbc + bl,
                       [[WIN * nbc, 128], [nbc, 128]]),
                    IDN[:])
                k0 = beta * ACCW + bb * 128 + j
                nc.vector.scalar_tensor_tensor(
                    ACC[:, k0:k0 + 128], ps[:], WIN4[:, j:j + 1],
                    ACC[:, k0:k0 + 128], op0=mult, op1=add)

            # wavefront step for one chain
            def wf_step(ch, g):
                eng, nbc, SR, Lm = ch["eng"], ch["nbc"], ch["SR"], ch["Lm"]
                rt = ch["ring"][:].tensor
                Ct = ch["C"][:].tensor
                RWT = (S + 1) * RNG * nbc
                s_lo = max(1, g - (WIN - 2))   # g-510
                s_hi = min(S, g + 1)
                ns = s_hi - s_lo + 1
                L = ns * nbc
                pp_ = (g - 1) % RNG
                pc = g % RNG
                pn = (g + 1) % RNG
                pr = sp.tile([128, 2 * Lm], ch["dt"], tag=f"pr{ch['nm']}",
                             name=f"pr{ch['nm']}")
                t2 = sp.tile([128, Lm], ch["dt"], tag=f"t2{ch['nm']}",
                             name=f"t2{ch['nm']}")
                prt = pr[:].tensor
                t2t = t2[:].tensor
                co = (s_lo - 1) * nbc
                if pc != 0:
                    eng.tensor_tensor(
                        AP(prt, 0, [[2 * Lm, 128], [Lm, 2], [nbc, ns], [1, nbc]]),
                        AP(rt, s_lo * SR + pp_ * nbc,
                           [[RWT, 128], [nbc, 2], [SR, ns], [1, nbc]]),
                        AP(Ct, co, [[2 * Lm, 128], [Lm, 2], [nbc, ns], [1, nbc]]),
                        op=mult)
                else:  # ring wrap between slabs g-1 (pos 15) and g (pos 0)
                    eng.tensor_tensor(
                        AP(prt, 0, [[2 * Lm, 128], [nbc, ns], [1, nbc]]),
                        AP(rt, s_lo * SR + pp_ * nbc,
                           [[RWT, 128], [SR, ns], [1, nbc]]),
                        AP(Ct, co, [[2 * Lm, 128], [1, L]]),
                        op=mult)
                    eng.tensor_tensor(
                        AP(prt, Lm, [[2 * Lm, 128], [nbc, ns], [1, nbc]]),
                        AP(rt, s_lo * SR + pc * nbc,
                           [[RWT, 128], [SR, ns], [1, nbc]]),
                        AP(Ct, Lm + co, [[2 * Lm, 128], [1, L]]),
                        op=mult)
                yield
                eng.tensor_tensor(
                    AP(t2t, 0, [[Lm, 128], [1, L]]),
                    AP(prt, 0, [[2 * Lm, 128], [1, L]]),
                    AP(prt, Lm, [[2 * Lm, 128], [1, L]]),
                    op=add)
                yield
                eng.tensor_tensor(
                    AP(rt, s_lo * SR + pn * nbc,
                       [[RWT, 128], [SR, ns], [1, nbc]]),
                    AP(t2t, 0, [[Lm, 128], [1, L]]),
                    AP(rt, (s_lo - 1) * SR + pc * nbc,
                       [[RWT, 128], [SR, ns], [1, nbc]]),
                    op=add)
                yield

            # ---- preamble staging ----
            issue_chunk(0)
            issue_chunk(1)
            stage_batch(0)
            stage_batch(8)

            # epilogue burst schedule: (g -> op) for j = 0..2
            epi_sched = {}
            for j in range(3):
                g0 = 128 * (j + 1) + 16
                for i, b in enumerate(range(NBK)):
                    epi_sched[g0 + 2 * i] = (j, b)

            # ---- wavefront ----
            for g in range(NSTEP):
                if g in (120, 248):
                    issue_chunk(2 if g == 120 else 3)
                # stage slot-0 x for slabs [g+8, g+16): their ring positions
                # alias slabs [g-8, g), whose last slot-0 read (op3 u-read)
                # was at step g-1 — emitting at the top of step g is the
                # earliest safe point in same-engine program order.
                if g % 8 == 0 and 8 <= g and g + 8 < WIN:
                    stage_batch(g + 8)
                if g >= 19 and (g - 19) % 8 == 0:
                    export_batch(g - 19, 8)
                if g in epi_sched:
                    epi_op(*epi_sched[g])
                steps = [wf_step(ch, g) for ch in chains]
                for _ in range(3):
                    for it in steps:
                        next(it, None)

            # ---- drain: remaining exports, epilogue j=3 + output,
            # pipelined per batch row so PE/DVE/DMA overlap ----
            export_batch(504, 8)
            for beta in range(NB):
                for b in range(8 * beta, 8 * beta + 8):
                    epi_op(3, b)
                for i, k0 in enumerate(K0S):
                    ps = pp.tile([128, 128], f32, tag="pso", name="pso")
                    nc.tensor.transpose(
                        ps[:], ACC[:, beta * ACCW + k0:beta * ACCW + k0 + 128],
                        IDN[:])
                    ot = sp.tile([128, 128], f32, tag="ot", name="ot")
                    nc.vector.tensor_tensor(ot[:], ps[:],
                                            RNT[:, i * 128:(i + 1) * 128], op=mult)
                    o_t = ot[:].tensor
                    if i == 0:
                        nc.sync.dma_start(
                            AP(out.tensor, beta * T, [[1, 1], [1, 64]]),
                            AP(o_t, 64, [[128, 1], [1, 64]]))
                        nc.sync.dma_start(
                            AP(out.tensor, beta * T + 64, [[128, 127], [1, 128]]),
                            AP(o_t, 128, [[128, 127], [1, 128]]))
                    elif i < 8:
                        nc.sync.dma_start(
                            AP(out.tensor, beta * T + k0 * 128 - PAD,
                               [[128, 128], [1, 128]]),
                            AP(o_t, 0, [[128, 128], [1, 128]]))
                    else:
                        nc.sync.dma_start(
                            AP(out.tensor, beta * T + 1025 * 128 - PAD,
                               [[1, 1], [1, 64]]),
                            AP(o_t, 127 * 128, [[128, 1], [1, 64]]))

    nc.compile()
    return nc


def _host_prep(ex, gain, biquads):
    # per-core host tables; frame n = beta*F + f -> p = n % 128, b = n // 128
    f32 = np.float32
    a0 = biquads[..., 0].astype(f32)
    a1 = biquads[..., 1].astype(f32)
    a2 = biquads[..., 2].astype(f32)
    c1 = (-a1 / a0).astype(f32)          # [NB, F, S]
    c2 = (-a2 / a0).astype(f32)
    gain_eff = (gain.astype(f32) * np.prod((1.0 / a0).astype(f32), axis=-1)
                * f32(FSCL)).astype(f32)

    # [p, s, b]
    c1pb = c1.reshape(NBK, 128, S).transpose(1, 2, 0)
    c2pb = c2.reshape(NBK, 128, S).transpose(1, 2, 0)
    cts = {}
    for nm, _, nbc, b0, fp16 in CHAINS:
        t = np.concatenate(
            [c2pb[:, :, b0:b0 + nbc].reshape(128, S * nbc),
             c1pb[:, :, b0:b0 + nbc].reshape(128, S * nbc)], axis=1)
        cts[f"ct_{nm}"] = np.ascontiguousarray(
            t.astype(np.float16 if fp16 else f32))
    g = gain_eff.reshape(NBK, 128).T     # [p, b]
    cts["gbrep"] = np.ascontiguousarray(np.tile(g, (1, 8)))  # [p, r*32+b]
    return cts


def _host_consts():
    f32 = np.float32
    win = _hann(WIN).astype(f32)
    WIN4 = np.ascontiguousarray(win.reshape(4, 128).T)
    norm = np.zeros(LFULL, f32)
    idx = (np.arange(F)[:, None] * HOP + np.arange(WIN)[None, :]).reshape(-1)
    np.add.at(norm, idx, np.broadcast_to(win, (F, WIN)).reshape(-1))
    rn = np.zeros_like(norm)
    nz = norm != 0
    rn[nz] = (f32(1.0 / FSCL) / norm[nz]).astype(f32)  # un-scale fp16 here
    rn2 = rn.reshape(NCELL, 128)
    RNT = np.zeros((128, 9 * 128), f32)
    for i, k0 in enumerate(K0S):
        RNT[:, i * 128:(i + 1) * 128] = rn2[k0:k0 + 128, :]
    IDN = np.eye(128, dtype=f32)
    return WIN4, RNT, IDN


def make_in_maps(ex, gain, biquads):
    ex = np.asarray(ex, np.float32)
    gain = np.asarray(gain, np.float32)
    biquads = np.asarray(biquads, np.float32)
    WIN4, RNT, IDN = _host_consts()
    in_maps = []
    for ci in range(NCORE):
        sl = slice(ci * NB, (ci + 1) * NB)
        m = {"ex": np.ascontiguousarray(ex[sl]),
             "win4": WIN4, "rnt": RNT, "idn": IDN}
        m.update(_host_prep(ex[sl], gain[sl], biquads[sl]))
        in_maps.append(m)
    return in_maps


def kernel(ex, gain, biquads):
    from concourse.bass_utils import run_bass_kernel_spmd

    if "nc" not in _CACHE:
        _CACHE["nc"] = _build_module()
    nc = _CACHE["nc"]
    in_maps = make_in_maps(ex, gain, biquads)
    res = run_bass_kernel_spmd(nc, in_maps, list(range(NCORE)))
    out = np.concatenate([res.results[ci]["out"] for ci in range(NCORE)], axis=0)
    return out.astype(np.float32)
